# revision 2
# baseline (speedup 1.0000x reference)
"""Trainium2 Bass kernel for a 2-layer GCN (CompressedGNN) — v2.

Computation (see reference):
    h1 = relu(A_hat @ (x @ W1) + b1)
    h2 = A_hat @ (h1 @ W2) + b2
    out = h2 @ Wc + bc
with A_hat = D^-1/2 (A + I) D^-1/2 built from edge_index (multi-edges kept).

Strategy (8 NeuronCores, SPMD, one program):
  - dst nodes grouped into 79 blocks of 128 (+1 dummy); blocks rank-sorted by
    edge count and dealt 8-per-position so all cores run an identical,
    balanced loop.  Node rows are permuted to (core, position, offset) order;
    an AllGather concatenation makes the full feature table addressable by
    permuted row id.
  - per layer: local feature matmul -> fp8(e4m3) store -> AllGather of the
    1280-row shard -> per position: dma_gather of the block's (padded) edge
    messages in fp8 and one PE matmul per 128-edge chunk with
    S[edge, dst_local] = fp8 edge weight, accumulating into PSUM.
  - self-loops are NOT in the edge list: each position's local (pre-gather)
    activations are kept in SBUF and folded in with one diagonal matmul
    (lhsT = diag(dinv^2)), which trims ~6% of gather DMA + PE chunks.
  - bias via a K=1 ones x bias matmul; relu on ACT during PSUM->SBUF; PE
    transposes produce the next layer's stationary operand in per-position
    tiles so the next layer's matmul interleaves into the aggregation loop.
  - classifier is interleaved into the layer-2 aggregation; host
    reassembles/unpermutes rows.

K-major layout convention: logical [512, M] operands live in SBUF/DRAM as
[128, 4*M] with contraction-chunk k occupying columns [k*M, (k+1)*M).
"""

import hashlib
import os

import numpy as np
import ml_dtypes

import concourse.bacc as bacc
import concourse.mybir as mybir
import concourse.tile as tile
from concourse.masks import make_identity
from concourse.bass_utils import run_bass_kernel_spmd

N_NODES = 10000
D_IN = 512
D_HID = 512
D_OUT = 100
NCORES = 8
P = 128
KC = D_HID // P            # 4 contraction chunks
NPOS = 10                  # dst-block positions per core
NP_CORE = NPOS * P         # 1280 padded rows per core
NTOT = NCORES * NP_CORE    # 10240 rows in the gathered table
NBLK = (N_NODES + P - 1) // P   # 79 real blocks
HALF = NPOS // 2           # positions per AllGather half
HROWS = NCORES * HALF * P  # rows in each gather-table half

GSUB = 12                  # chunks per dma_gather
NMT = 4                    # rotated gather buffers
LOOKAHEAD = 4              # early-gather emission lead over consumption

bf16 = mybir.dt.bfloat16
f32 = mybir.dt.float32
i16 = mybir.dt.int16

# e3m4 carries one more mantissa bit than e4m3 (half the quantization
# error) at the cost of range ([2^-2, 15.5] normal), so S and the messages
# are pre-scaled into its sweet spot and unscaled during the PSUM->SBUF
# activation; e4m3 would instead allow DoubleRow matmuls (2x PE) but the
# extra quantization error eats too much of the correctness budget
USE_E3 = False
fp8 = mybir.dt.float8e3 if USE_E3 else mybir.dt.float8e4
np_fp8 = ml_dtypes.float8_e3m4 if USE_E3 else ml_dtypes.float8_e4m3

_COMPILED = {}


def _kmajor(a):
    """[KC*P, M] -> [P, KC*M] with chunk k at columns [k*M, (k+1)*M)."""
    km, m = a.shape
    assert km == KC * P
    return np.concatenate([a[k * P:(k + 1) * P] for k in range(KC)], axis=1)


# ----------------------------------------------------------------------------
# host-side preprocessing
# ----------------------------------------------------------------------------

def _preprocess(x, edge_index, W1, b1, W2, b2, Wc, bc):
    src = np.asarray(edge_index[0], dtype=np.int64)
    dst = np.asarray(edge_index[1], dtype=np.int64)
    n = N_NODES

    deg = 1.0 + np.bincount(dst, minlength=n).astype(np.float64)
    dinv = 1.0 / np.sqrt(deg)
    w = (dinv[src] * dinv[dst]).astype(np.float32)
    w_self = (dinv * dinv).astype(np.float32)

    def pow2_scale(absmax, cap=12.0):
        return float(2.0 ** np.floor(np.log2(cap / max(absmax, 1e-30))))

    if USE_E3:
        ssc = pow2_scale(max(w.max(), w_self.max()))
        # message magnitudes, computed once on the host to pick safe
        # per-layer power-of-two scales (inputs are fixed by the harness)
        x32f = np.asarray(x, np.float32)
        h1m = x32f @ np.asarray(W1, np.float32)
        m1 = pow2_scale(np.abs(h1m).max())
        agg1 = np.zeros_like(h1m)
        np.add.at(agg1, dst, w[:, None] * h1m[src])
        agg1 += w_self[:, None] * h1m
        h1f = np.maximum(agg1 + np.asarray(b1, np.float32), 0.0)
        h2m = h1f @ np.asarray(W2, np.float32)
        m2 = pow2_scale(np.abs(h2m).max())
        del h1m, agg1, h1f, h2m
    else:
        ssc, m1, m2 = 1.0, 1.0, 1.0
    w = w * ssc
    w_self = w_self * ssc

    blk = dst // P
    cnt = np.bincount(blk, minlength=NBLK)
    # a block's gather rows are the UNIQUE sources of its edges (a repeated
    # source is fetched once; its S column simply has several nonzeros)
    upair = np.unique(blk * n + src)
    ucnt = np.bincount((upair // n).astype(np.int64), minlength=NBLK)
    c_b = np.maximum(1, -(-ucnt // P))                   # chunks per block
    c_b80 = np.concatenate([c_b, [1]])                   # dummy block 79
    ranks = np.argsort(-c_b80, kind="stable")
    assign = ranks.reshape(NPOS, NCORES)                 # [pos, core] -> block

    # permuted row id for every node (core-major: core, position, offset) —
    # used to reassemble the per-core outputs
    permrow = np.zeros(n, dtype=np.int64)
    # gather-table row id, split across two tensors: positions 0..HALF-1 of
    # all cores form table A, the rest table B (B rows are B-local).  The
    # AllGather then splits into two collectives, and gathers from table A
    # can start as soon as the first one lands.
    permtab = np.zeros(n, dtype=np.int64)
    for j in range(NPOS):
        for k in range(NCORES):
            b = assign[j, k]
            if b >= NBLK:
                continue
            lo, hi = b * P, min((b + 1) * P, n)
            nodes = np.arange(lo, hi)
            permrow[nodes] = k * NP_CORE + j * P + (nodes - lo)
            permtab[nodes] = ((j >= HALF) * HROWS + k * HALF * P
                              + (j % HALF) * P + (nodes - lo))

    # edges grouped by dst block
    eorder = np.argsort(blk, kind="stable")
    estart = np.concatenate([[0], np.cumsum(cnt)]).astype(int)

    src_perm = permtab[src]                              # gather row ids

    # per-block unique-source rows, split at the A/B table boundary
    blocks = [None] * NBLK
    for b in range(NBLK):
        if cnt[b] == 0:
            continue
        e = eorder[estart[b]:estart[b + 1]]
        urows, inv = np.unique(src_perm[e], return_inverse=True)
        nlow = int(np.searchsorted(urows, HROWS))
        dloc = (dst[e] - b * P).astype(np.int64)
        blocks[b] = (urows, inv, dloc, w[e], nlow)

    # chunk counts per position, per table side (uniform across cores)
    C_A, C_Bh = [], []
    for j in range(NPOS):
        ca = cb2 = 0
        for k in range(NCORES):
            b = assign[j, k]
            if b >= NBLK or blocks[b] is None:
                continue
            urows, inv, dloc, we, nlow = blocks[b]
            ca = max(ca, -(-nlow // P))
            cb2 = max(cb2, -(-(len(urows) - nlow) // P))
        if ca + cb2 == 0:
            ca = 1
        C_A.append(ca)
        C_Bh.append(cb2)
    C_B = [C_A[j] + C_Bh[j] for j in range(NPOS)]
    split = C_A
    chunk_off = np.concatenate([[0], np.cumsum(C_B)]).astype(int)
    total_chunks = int(chunk_off[-1])

    S_cores = []
    idx_cores = []
    xT_cores = []
    Sself_cores = []
    x32 = np.asarray(x, dtype=np.float32)
    for k in range(NCORES):
        S_k = np.zeros((P, total_chunks * P), dtype=np_fp8)
        idx_k = np.zeros((P, total_chunks * 8), dtype=np.int16)
        xT_k = np.zeros((D_IN, NP_CORE), dtype=np.float32)
        Ss_k = np.zeros((P, NPOS * P), dtype=np_fp8)
        for j in range(NPOS):
            b = assign[j, k]
            cap = C_B[j] * P
            capa = C_A[j] * P
            off = int(chunk_off[j])
            idxl = np.zeros(cap, dtype=np.int16)
            if b < NBLK and blocks[b] is not None:
                urows, inv, dloc, we, nlow = blocks[b]
                nu = len(urows)
                nhi = nu - nlow
                assert nlow <= capa and nhi <= cap - capa
                S2 = np.zeros((nu, P), dtype=np.float32)
                np.add.at(S2, (inv, dloc), we)
                Spad = np.zeros((cap, P), dtype=np.float32)
                Spad[:nlow] = S2[:nlow]
                Spad[capa:capa + nhi] = S2[nlow:]
                idxl[:nlow] = urows[:nlow].astype(np.int16)
                idxl[capa:capa + nhi] = (urows[nlow:]
                                         - HROWS).astype(np.int16)
                S_k[:, off * P:(off + C_B[j]) * P] = (
                    Spad.reshape(C_B[j], P, P).transpose(1, 0, 2)
                    .reshape(P, C_B[j] * P).astype(np_fp8))
            if b < NBLK:
                lo, hi = b * P, min((b + 1) * P, n)
                xT_k[:, j * P:j * P + (hi - lo)] = x32[lo:hi].T
                nv = hi - lo
                Ss_k[np.arange(nv), j * P + np.arange(nv)] = (
                    w_self[lo:hi].astype(np_fp8))
            # column-major 16-wrap layout, replicated to 128 partitions
            idx_k[:, off * 8:(off + C_B[j]) * 8] = np.tile(
                idxl.reshape(-1, 16).T, (8, 1))
        S_cores.append(S_k)
        idx_cores.append(idx_k)
        Sself_cores.append(Ss_k)
        xT_cores.append(_kmajor(xT_k).astype(ml_dtypes.bfloat16))

    weights = {
        "W1": _kmajor(np.asarray(W1, np.float32)).astype(ml_dtypes.bfloat16),
        "W2": _kmajor(np.asarray(W2, np.float32)).astype(ml_dtypes.bfloat16),
        "Wc": _kmajor(np.asarray(Wc, np.float32)).astype(ml_dtypes.bfloat16),
        # layer biases enter the PSUM accumulator, which carries the S and
        # message scales — pre-scale them to match; the classifier bias is
        # added at true scale
        "b1": (np.asarray(b1, np.float32)
               * ssc * m1).astype(ml_dtypes.bfloat16)[None, :],
        "b2": (np.asarray(b2, np.float32)
               * ssc * m2).astype(ml_dtypes.bfloat16)[None, :],
        "bc": np.asarray(bc, np.float32).astype(ml_dtypes.bfloat16)[None, :],
    }
    return {
        "C_B": tuple(C_B),
        "split": tuple(split),
        "scales": (ssc, m1, m2),
        "total_chunks": total_chunks,
        "chunk_off": chunk_off,
        "permrow": permrow,
        "S_cores": S_cores,
        "idx_cores": idx_cores,
        "Sself_cores": Sself_cores,
        "xT_cores": xT_cores,
        "weights": weights,
    }


# ----------------------------------------------------------------------------
# device program
# ----------------------------------------------------------------------------

def _build(C_B, total_chunks, chunk_off, split=None, scales=(1.0, 1.0, 1.0),
           spmd=True, unroll=1,
           skip_gather=False, skip_cc=False, skip_smm=False,
           skip_trans=False, gsub=GSUB, tag=0):
    if split is None:
        split = tuple(0 for _ in C_B)
    ssc, m1, m2 = scales
    nc = bacc.Bacc("TRN2", target_bir_lowering=False, debug=False,
                   num_devices=NCORES if spmd else 1, num_swdge_queues=2,
                   dynamic_dma_scratch_size=40960)

    xT_d = nc.dram_tensor("xT", [P, KC * NP_CORE], bf16, kind="ExternalInput")
    W1_d = nc.dram_tensor("W1", [P, KC * D_HID], bf16, kind="ExternalInput")
    W2_d = nc.dram_tensor("W2", [P, KC * D_HID], bf16, kind="ExternalInput")
    Wc_d = nc.dram_tensor("Wc", [P, KC * D_OUT], bf16, kind="ExternalInput")
    b1_d = nc.dram_tensor("b1", [1, D_HID], bf16, kind="ExternalInput")
    b2_d = nc.dram_tensor("b2", [1, D_HID], bf16, kind="ExternalInput")
    bc_d = nc.dram_tensor("bc", [1, D_OUT], bf16, kind="ExternalInput")
    S_d = nc.dram_tensor("S", [P, total_chunks * P], fp8,
                         kind="ExternalInput")
    Ss_d = nc.dram_tensor("Sself", [P, NPOS * P], fp8, kind="ExternalInput")
    idx_d = nc.dram_tensor("idx", [P, total_chunks * 8], i16,
                           kind="ExternalInput")
    out_d = nc.dram_tensor("out", [NP_CORE, D_OUT], f32,
                           kind="ExternalOutput")
    # distinct I/O signature per program variant: the PJRT-level NEFF cache
    # key does not cover the embedded BIR, so two different programs with
    # identical I/O would collide
    tag_d = nc.dram_tensor("vtag", [1, 8 * (tag + 1)], f32,
                           kind="ExternalInput")
    tagout_d = nc.dram_tensor("vtagout", [1, 8 * (tag + 1)], f32,
                              kind="ExternalOutput")

    with tile.TileContext(nc) as tc:
        with (
            tc.tile_pool(name="const", bufs=1) as cpool,
            tc.tile_pool(name="sbuf", bufs=3) as sb,
            tc.tile_pool(name="gath", bufs=1) as gp,
            tc.tile_pool(name="psum", bufs=2, space="PSUM") as ps,
            tc.tile_pool(name="psum_tr", bufs=2, space="PSUM") as pst,
            tc.tile_pool(name="psum_mm", bufs=3, space="PSUM") as psm,
            tc.tile_pool(name="psum_warm", bufs=1, space="PSUM") as psw,
            tc.tile_pool(name="dram", bufs=1, space="DRAM") as dr,
            tc.tile_pool(name="dram_sh", bufs=1, space="DRAM") as drs,
        ):
            # ---------------- constants / resident tensors ----------------
            def load_const(name, dram, shape, dtype):
                t = cpool.tile(shape, dtype, tag=name)
                nc.sync.dma_start(t[:], dram[:])
                return t

            # load order matters: the first feature matmul only needs xT+W1,
            # so they go first; everything else overlaps the matmul phase
            xT_sb = load_const("xT", xT_d, [P, KC * NP_CORE], bf16)
            W1_sb = load_const("w1", W1_d, [P, KC * D_HID], bf16)
            S_sb = load_const("S", S_d, [P, total_chunks * P], fp8)
            idx_sb = load_const("idx", idx_d, [P, total_chunks * 8], i16)
            Ss_sb = load_const("Ss", Ss_d, [P, NPOS * P], fp8)
            b1_sb = load_const("b1", b1_d, [1, D_HID], bf16)
            W2_sb = load_const("w2", W2_d, [P, KC * D_HID], bf16)
            b2_sb = load_const("b2", b2_d, [1, D_HID], bf16)
            Wc_sb = load_const("wc", Wc_d, [P, KC * D_OUT], bf16)
            bc_sb = load_const("bc", bc_d, [1, D_OUT], bf16)
            tag_sb = load_const("vtag", tag_d, [1, 8 * (tag + 1)], f32)
            nc.sync.dma_start(tagout_d[:], tag_sb[:])

            ones_sb = cpool.tile([1, P], bf16, tag="ones")
            nc.vector.memset(ones_sb[:], 1.0)
            ident = cpool.tile([P, P], bf16, tag="ident")
            make_identity(nc, ident[:])

            # zero fp32 operands for PE warm-up matmuls: the tensor engine
            # drops to a low clock after idling (HAM), so dummy matmuls keep
            # it hot across the AllGather windows where it has no real work
            zf_l = cpool.tile([P, P], f32, tag="zfl")
            zf_r = cpool.tile([P, D_HID], f32, tag="zfr")
            nc.gpsimd.memset(zf_l[:], 0.0)
            nc.gpsimd.memset(zf_r[:], 0.0)

            def warm(n):
                for _ in range(n):
                    wp = psw.tile([P, D_HID], f32, space="PSUM", tag="warm")
                    nc.tensor.matmul(out=wp[:], lhsT=zf_l[:], rhs=zf_r[:],
                                     start=True, stop=True)

            # per-position transposed activations (next layer's stationary
            # operand) and local pre-aggregation activations (self loops)
            hT = [cpool.tile([P, KC * P], bf16, tag=f"hT{j}",
                             name=f"hT{j}") for j in range(NPOS)]
            # per-position pre-aggregation activations, already in fp8: one
            # ACT copy from PSUM feeds the g_loc store AND the self-loop
            # matmul, so PSUM slots recycle after a single reader
            hq = [cpool.tile([P, D_HID], fp8, tag=f"hq{j}",
                             name=f"hq{j}") for j in range(NPOS)]
            if skip_trans:
                for j in range(NPOS):
                    nc.vector.memset(hT[j][:], 0.0)

            # per layer, the local shard is stored as two half tensors so
            # the AllGather splits in two: the first half fires as soon as
            # positions 0..HALF-1 are stored and overlaps remaining compute
            g_loc = [(dr.tile([HALF * P, D_HID], fp8, tag=f"gla{i}",
                              name=f"gla{i}"),
                      dr.tile([HALF * P, D_HID], fp8, tag=f"glb{i}",
                              name=f"glb{i}")) for i in range(2 * unroll)]
            g_full = [(drs.tile([HROWS, D_HID], fp8,
                                addr_space="Shared" if spmd else "Local",
                                tag=f"gfa{i}", name=f"gfa{i}"),
                       drs.tile([HROWS, D_HID], fp8,
                                addr_space="Shared" if spmd else "Local",
                                tag=f"gfb{i}", name=f"gfb{i}"))
                      for i in range(2 * unroll)]

            # gather lanes are fully written before any matmul reads them
            # (pad indices fetch row 0; trimmed lanes are never multiplied),
            # so the buffers need no clearing
            mts = [gp.tile([P, gsub, D_HID], fp8, tag=f"mt{i}", name=f"mt{i}")
                   for i in range(NMT)]
            # dedicated per-position buffers for the early gathers (rows all
            # in the lower table half), issued for every position up front
            emts = [gp.tile([P, max(split[j], 1), D_HID], fp8,
                            tag=f"emt{j}", name=f"emt{j}")
                    for j in range(NPOS)]
            state = {"g": 0}

            def mm_pos(j, lhsT_cols, rhs_big, n_out, bias=None,
                       store_dram=None, store_row=0, store_scale=1.0,
                       out_ext=None):
                """One position's feature matmul.

                lhsT_cols(k) -> AP of the [P, P] stationary chunk k.
                """
                acc = psm.tile([P, n_out], f32, space="PSUM", tag="mm")
                for k in range(KC):
                    nc.tensor.matmul(
                        out=acc[:], lhsT=lhsT_cols(k),
                        rhs=rhs_big[:, k * n_out:(k + 1) * n_out],
                        start=(k == 0),
                        stop=(k == KC - 1 and bias is None),
                    )
                if bias is not None:
                    nc.tensor.matmul(out=acc[:], lhsT=ones_sb[:],
                                     rhs=bias[:], start=False, stop=True)
                if store_dram is not None:
                    nc.scalar.activation(
                        hq[j][:], acc[:], mybir.ActivationFunctionType.Copy,
                        scale=store_scale)
                    nc.sync.dma_start(
                        store_dram[store_row:store_row + P, :], hq[j][:])
                if out_ext is not None:
                    o32 = sb.tile([P, n_out], f32, tag="o32")
                    nc.vector.tensor_copy(o32[:], acc[:])
                    nc.sync.dma_start(out_ext[j * P:(j + 1) * P, :], o32[:])

            def smm_group(acc, mt_t, mt_base, c_base, g, first):
                """S-matmuls for chunks [c_base, c_base+g) of one block,
                reading gather lanes mt_t[:, mt_base:mt_base+g, :].

                DoubleRow fp8: one instruction contracts a pair of
                consecutive 128-edge chunks (2x PE throughput); the S
                block-column layout already pairs them in memory."""
                if USE_E3:
                    for c in range(g):
                        nc.tensor.matmul(
                            out=acc[:],
                            lhsT=S_sb[:, (c_base + c) * P:
                                      (c_base + c + 1) * P],
                            rhs=mt_t[:, mt_base + c, :],
                            start=(first and c == 0), stop=False,
                        )
                    return
                for cp in range(g // 2):
                    a = c_base + 2 * cp
                    nc.tensor.matmul(
                        out=acc[:],
                        lhsT=S_sb[:, a * P:(a + 2) * P].rearrange(
                            "p (two f) -> p two f", two=2),
                        rhs=mt_t[:, mt_base + 2 * cp:mt_base + 2 * cp + 2, :],
                        start=(first and cp == 0), stop=False,
                        perf_mode=mybir.MatmulPerfMode.DoubleRow,
                    )
                if g % 2:
                    c = c_base + g - 1
                    nc.tensor.matmul(
                        out=acc[:],
                        lhsT=S_sb[:, c * P:(c + 1) * P],
                        rhs=mt_t[:, mt_base + g - 1, :],
                        start=(first and g == 1), stop=False,
                    )

            def agg_early(j, g_full_t):
                """Gather a position's table-A chunks — they only wait for
                the first half-collective."""
                sc = split[j]
                if sc == 0 or skip_gather:
                    return
                off = int(chunk_off[j])
                gi = state["g"]; state["g"] += 1
                nc.gpsimd.dma_gather(
                    emts[j][:, :sc, :], g_full_t[0][:],
                    idx_sb[:, off * 8:(off + sc) * 8],
                    sc * P, sc * P, D_HID,
                    queue_num=gi % 2,
                )

            def agg_pos(layer, j, g_full_t, bias, relu, act_scale=1.0):
                """One position: gather messages, S-matmul, self, bias, act,
                transpose into hT[j]."""
                cb = C_B[j]
                sc = split[j] if not skip_gather else 0
                off = int(chunk_off[j])
                acc = ps.tile([P, D_HID], f32, space="PSUM", tag="agg")
                if sc and not skip_smm:
                    smm_group(acc, emts[j], 0, off, sc, True)
                for c0 in range(sc, cb, gsub):
                    g = min(gsub, cb - c0)
                    gi = state["g"]; state["g"] += 1
                    mt = mts[gi % NMT]
                    if not skip_gather:
                        nc.gpsimd.dma_gather(
                            mt[:, :g, :], g_full_t[1][:],
                            idx_sb[:, (off + c0) * 8:(off + c0 + g) * 8],
                            g * P, g * P, D_HID,
                            queue_num=gi % 2,
                        )
                    if not skip_smm:
                        smm_group(acc, mt, 0, off + c0, g, c0 == sc == 0)
                # self-loop: diag(dinv^2) @ local pre-aggregation rows
                nc.tensor.matmul(out=acc[:],
                                 lhsT=Ss_sb[:, j * P:(j + 1) * P],
                                 rhs=hq[j][:],
                                 start=skip_smm, stop=False)
                nc.tensor.matmul(out=acc[:], lhsT=ones_sb[:], rhs=bias[:],
                                 start=False, stop=True)
                h_sb = sb.tile([P, D_HID], bf16, tag=f"h{layer}")
                nc.scalar.activation(
                    h_sb[:], acc[:],
                    mybir.ActivationFunctionType.Relu if relu
                    else mybir.ActivationFunctionType.Copy,
                    scale=act_scale)
                # transposes for the next matmul's stationary operand
                if not skip_trans:
                    for k in range(KC):
                        tp = pst.tile([P, P], bf16, space="PSUM", tag="tr")
                        nc.tensor.transpose(
                            out=tp[:], in_=h_sb[:, k * P:(k + 1) * P],
                            identity=ident[:])
                        nc.vector.tensor_copy(hT[j][:, k * P:(k + 1) * P],
                                              tp[:])

            def allgather(i, h):
                """Gather half h (positions h*HALF..) into table h."""
                if skip_cc:
                    return
                dst = g_full[i][h]
                if not spmd:
                    # single-core timing/sim variant: stand in for the
                    # collective with equivalent local DRAM traffic
                    for r in range(NCORES):
                        nc.sync.dma_start(
                            dst[r * HALF * P:(r + 1) * HALF * P, :],
                            g_loc[i][h][:])
                    return
                nc.gpsimd.collective_compute(
                    "AllGather",
                    mybir.AluOpType.bypass,
                    replica_groups=[list(range(NCORES))],
                    ins=[g_loc[i][h].opt()],
                    outs=[dst.opt()],
                )

            def store_half(i, j):
                return g_loc[i][j // HALF], (j % HALF) * P

            warm(3)   # cover the initial xT/W1 load window
            for _rep in range(unroll):
                # ---------------- layer 1 feature matmul ----------------
                for j in range(NPOS):
                    tgt, row = store_half(2 * _rep, j)
                    mm_pos(j, lambda k, j=j: xT_sb[:, k * NP_CORE + j * P:
                                                   k * NP_CORE + (j + 1) * P],
                           W1_sb, D_HID, store_dram=tgt, store_row=row,
                           store_scale=m1)
                    if j == HALF - 1:
                        allgather(2 * _rep, 0)
                allgather(2 * _rep, 1)
                # stagger the early-gather descriptor generation: an upfront
                # pass of all ten would hog the serial Q7 generator and
                # starve the late gathers the first positions need next
                for j in range(LOOKAHEAD):
                    agg_early(j, g_full[2 * _rep])
                warm(10)

                # ------- layer 1 aggregate + layer 2 matmul, per position ---
                for j in range(NPOS):
                    agg_pos(1, j, g_full[2 * _rep], b1_sb, True,
                            act_scale=1.0 / (ssc * m1))
                    if j + LOOKAHEAD < NPOS:
                        agg_early(j + LOOKAHEAD, g_full[2 * _rep])
                    tgt, row = store_half(2 * _rep + 1, j)
                    mm_pos(j, lambda k, j=j: hT[j][:, k * P:(k + 1) * P],
                           W2_sb, D_HID, store_dram=tgt, store_row=row,
                           store_scale=m2)
                    if j == HALF - 1:
                        allgather(2 * _rep + 1, 0)
                allgather(2 * _rep + 1, 1)
                for j in range(LOOKAHEAD):
                    agg_early(j, g_full[2 * _rep + 1])
                warm(10)

                # ------- layer 2 aggregate + classifier, per position -------
                for j in range(NPOS):
                    agg_pos(2, j, g_full[2 * _rep + 1], b2_sb, False,
                            act_scale=1.0 / (ssc * m2))
                    if j + LOOKAHEAD < NPOS:
                        agg_early(j + LOOKAHEAD, g_full[2 * _rep + 1])
                    mm_pos(j, lambda k, j=j: hT[j][:, k * P:(k + 1) * P],
                           Wc_sb, D_OUT, bias=bc_sb, out_ext=out_d)

    nc.compile()
    return nc


def _in_maps(pre, tag=0):
    wts = pre["weights"]
    vtag = np.zeros((1, 8 * (tag + 1)), np.float32)
    maps = []
    for k in range(NCORES):
        maps.append({
            "xT": pre["xT_cores"][k],
            "W1": wts["W1"], "W2": wts["W2"], "Wc": wts["Wc"],
            "b1": wts["b1"], "b2": wts["b2"], "bc": wts["bc"],
            "S": pre["S_cores"][k],
            "Sself": pre["Sself_cores"][k],
            "idx": pre["idx_cores"][k],
            "vtag": vtag,
        })
    return maps


def _get_program(C_B, total_chunks, chunk_off, split, scales=(1.0, 1.0, 1.0)):
    key = (tuple(C_B), tuple(split), tuple(scales))
    if key not in _COMPILED:
        _COMPILED[key] = _build(C_B, total_chunks, chunk_off, split=split,
                                scales=scales)
    return _COMPILED[key]


# ----------------------------------------------------------------------------
# entry point
# ----------------------------------------------------------------------------

def kernel(x, edge_index, W1, b1, W2, b2, Wc, bc, _want_trace=False,
           **trace_kwargs):
    pre = _preprocess(x, edge_index, W1, b1, W2, b2, Wc, bc)
    nc = _get_program(pre["C_B"], pre["total_chunks"], pre["chunk_off"],
                      pre["split"], pre["scales"])

    # key the neuronx NEFF cache by program content: the PJRT-level module
    # fingerprint used as the cache key does not cover the embedded BIR, so
    # two programs with identical I/O signatures would otherwise collide
    if "NEURON_COMPILE_CACHE_URL" not in os.environ:
        sha = hashlib.sha256(nc.to_json_bytes()).hexdigest()[:16]
        os.environ["NEURON_COMPILE_CACHE_URL"] = f"/tmp/neuron-cache-{sha}"

    in_maps = _in_maps(pre)

    res = run_bass_kernel_spmd(nc, in_maps, core_ids=list(range(NCORES)),
                               trace=_want_trace, **trace_kwargs)

    big = np.concatenate([res.results[k]["out"] for k in range(NCORES)],
                         axis=0)
    out = big[pre["permrow"]].astype(np.float32)
    if _want_trace:
        return out, res
    return out
